# revision 5
# baseline (speedup 1.0000x reference)
import sys

sys.path.insert(0, "/opt/trn_rl_repo")

import numpy as np

import concourse.bass as bass
import concourse.tile as tile
from concourse import bacc, mybir
from concourse.bass_utils import run_bass_kernel_spmd

# Model dims (hardcoded per contract)
B, S, W = 64, 128, 16
CDIM, NF = 50, 50
WDIM = 300
KS = [3, 4, 5]
HID, NLAB = 256, 5
COMB = WDIM + NF * 3  # 450
NCORES = 8
BL = B // NCORES  # 8 seqs per core
NTOK = BL * S  # 1024 tokens per core, col index = t*BL + b
NW = 21  # padded char positions (-2..18)

F32 = mybir.dt.float32
BF16 = mybir.dt.bfloat16
NP_BF16 = mybir.dt.np(BF16)

_CACHE = {}


def _gate_perm():
    # torch gate order i,f,g,o -> our chunk order i0 i1 f0 f1 o0 o1 g0 g1
    idx = np.arange(4 * HID).reshape(4, HID)  # i,f,g,o
    return np.concatenate([idx[0], idx[1], idx[3], idx[2]])  # i,f,o,g


def build_nc():
    nc = bacc.Bacc("TRN2", target_bir_lowering=False, debug=False,
                   num_devices=NCORES)

    # ---- DRAM inputs ----
    d_ce2 = nc.dram_tensor("ce2", [100, NW * NTOK], BF16, kind="ExternalInput")
    d_xwe = nc.dram_tensor("xwe", [384, NTOK], BF16, kind="ExternalInput")
    d_convA = nc.dram_tensor("convA", [100, 5 * 64], BF16, kind="ExternalInput")
    d_convB = nc.dram_tensor("convB", [50, 2 * 64], BF16, kind="ExternalInput")
    d_convbias = nc.dram_tensor("convbias", [50, 3], F32, kind="ExternalInput")
    d_wih0 = nc.dram_tensor("wih0", [128, 2 * 4096], BF16, kind="ExternalInput")
    d_wih1 = nc.dram_tensor("wih1", [128, 2 * 4096], BF16, kind="ExternalInput")
    d_whh = nc.dram_tensor("whh", [128, 4 * 2048], BF16, kind="ExternalInput")
    d_bias = nc.dram_tensor("bias", [128, 4 * 8], F32, kind="ExternalInput")
    d_outw = nc.dram_tensor("outw", [128, 4 * 8], BF16, kind="ExternalInput")
    d_outb = nc.dram_tensor("outb", [5, 1], F32, kind="ExternalInput")
    d_crf = nc.dram_tensor("crfc", [5, 16], F32, kind="ExternalInput")
    # crfc cols: 0-4 expT rows(K=5,M=5), 5 start, 6 exp(end), 7 ones
    d_emmask = nc.dram_tensor("emmask", [5, NTOK], BF16, kind="ExternalInput")
    d_ohmask = nc.dram_tensor("ohmask", [5, NTOK], F32, kind="ExternalInput")
    d_numc = nc.dram_tensor("numc", [1, BL], F32, kind="ExternalInput")
    d_ones15 = nc.dram_tensor("ones15", [1, 5], F32, kind="ExternalInput")

    d_llh = nc.dram_tensor("llh", [1, BL], F32, kind="ExternalOutput")
    d_dbg_em = nc.dram_tensor("dbg_em", [5, NTOK], F32, kind="ExternalOutput")

    with tile.TileContext(nc) as tc:
        with (
            tc.tile_pool(name="big", bufs=1) as big,
            tc.tile_pool(name="wpool", bufs=1) as wp,
            tc.tile_pool(name="state", bufs=1) as st,
            tc.tile_pool(name="work", bufs=3) as wk,
            tc.tile_pool(name="pacc", bufs=2) as pk,
            tc.tile_pool(name="psA", bufs=2, space="PSUM") as psA,
            tc.tile_pool(name="psB", bufs=2, space="PSUM") as psB,
            tc.tile_pool(name="psC", bufs=2, space="PSUM") as psC,
            tc.tile_pool(name="psT", bufs=2, space="PSUM") as psT,
        ):
            # ---- load phase ----
            ce2 = big.tile([100, NW * NTOK], BF16, tag="ce2")
            nc.sync.dma_start(ce2[:], d_ce2[:])
            xt = [big.tile([128, NTOK], BF16, tag=f"x{i}", name=f"x{i}")
                  for i in range(4)]
            for i in range(3):
                nc.sync.dma_start(xt[i][:], d_xwe[i * 128:(i + 1) * 128, :])
            nc.vector.memset(xt[3][:], 0.0)

            convA = wp.tile([100, 5 * 64], BF16, tag="cA")
            convB = wp.tile([50, 2 * 64], BF16, tag="cB")
            convbias = wp.tile([50, 3], F32, tag="cb")
            nc.sync.dma_start(convA[:], d_convA[:])
            nc.sync.dma_start(convB[:], d_convB[:])
            nc.sync.dma_start(convbias[:], d_convbias[:])

            wih0 = wp.tile([128, 2 * 4096], BF16, tag="wih0")
            wih1 = wp.tile([128, 2 * 4096], BF16, tag="wih1")
            whh = wp.tile([128, 4 * 2048], BF16, tag="whh")
            bias = wp.tile([128, 4 * 8], F32, tag="bias")
            outw = wp.tile([128, 4 * 8], BF16, tag="outw")
            outb = wp.tile([5, 1], F32, tag="outb")
            crfc = wp.tile([5, 16], F32, tag="crfc")
            emmask = wp.tile([5, NTOK], BF16, tag="emmask")
            ohmask = wp.tile([5, NTOK], F32, tag="ohmask")
            numc = wp.tile([1, BL], F32, tag="numc")
            ones15 = wp.tile([1, 5], F32, tag="ones15")
            for dst, src in [(wih0, d_wih0), (wih1, d_wih1), (whh, d_whh),
                             (bias, d_bias), (outw, d_outw), (outb, d_outb),
                             (crfc, d_crf), (emmask, d_emmask),
                             (ohmask, d_ohmask), (numc, d_numc),
                             (ones15, d_ones15)]:
                nc.sync.dma_start(dst[:], src[:])

            # ---- char conv: 3 kernels, max over positions ----
            acc = [pk.tile([50, NTOK], F32, tag=f"acc{k}", name=f"acc{k}")
                   for k in range(3)]
            for a in acc:
                nc.vector.memset(a[:], 0.0)

            # (kernel_idx, n_positions, list of (lhsT_sel, q_offset))
            # lhsT_sel: ('A', col, rows) or ('B', col, rows)
            plans = [
                (0, 16, [(('A', 0, 100), -1), (('B', 0, 50), 1)]),
                (1, 17, [(('A', 64, 100), -2), (('A', 128, 100), 0)]),
                (2, 16, [(('A', 192, 100), -2), (('A', 256, 100), 0),
                         (('B', 64, 50), 2)]),
            ]
            for ki, npos, taps in plans:
                for p in range(npos):
                    for n in range(2):
                        ps = psA.tile([50, 512], F32, tag="cps")
                        for ti, (sel, qoff) in enumerate(taps):
                            which, col, rows = sel
                            lhsT = (convA if which == 'A' else convB)
                            lt = lhsT[0:rows, col:col + 50]
                            w = p + qoff + 2
                            rhs = ce2[0:rows, w * NTOK + n * 512:
                                      w * NTOK + n * 512 + 512]
                            nc.tensor.matmul(ps[:], lt, rhs,
                                             start=(ti == 0),
                                             stop=(ti == len(taps) - 1))
                        nc.vector.tensor_max(acc[ki][:, n * 512:(n + 1) * 512],
                                             acc[ki][:, n * 512:(n + 1) * 512],
                                             ps[:])
            # relu(acc + b) -> aligned char rows: k3->t2[64:114],
            # k4->t3[0:50], k5->t3[64:114] (Wih rows permuted to match)
            RL = mybir.ActivationFunctionType.Relu
            nc.scalar.activation(xt[2][64:114, :], acc[0][:],
                                 RL, bias=convbias[:, 0:1])
            nc.scalar.activation(xt[3][0:50, :], acc[1][:],
                                 RL, bias=convbias[:, 1:2])
            nc.scalar.activation(xt[3][64:114, :], acc[2][:],
                                 RL, bias=convbias[:, 2:3])

            # ---- helpers ----
            def proj(wih_t, src_tiles, xg_d, bias_col0):
                # xg[d][:, m*NTOK + t*BL+b] for d in 0,1
                for d in range(2):
                    for m in range(8):
                        for n in range(2):
                            ps = psB.tile([128, 512], F32, tag="proj")
                            for kc in range(4):
                                lt = wih_t[:, d * 4096 + kc * 1024 + m * 128:
                                           d * 4096 + kc * 1024 + m * 128 + 128]
                                rhs = src_tiles[kc][:, n * 512:(n + 1) * 512]
                                nc.tensor.matmul(ps[:], lt, rhs,
                                                 start=(kc == 0), stop=(kc == 3))
                            nc.vector.tensor_scalar_add(
                                xg_d[d][:, m * NTOK + n * 512:
                                        m * NTOK + n * 512 + 512],
                                ps[:],
                                bias[:, bias_col0 + d * 8 + m:
                                     bias_col0 + d * 8 + m + 1])

            xg = [big.tile([128, 8 * NTOK], BF16, tag=f"xg{d}", name=f"xg{d}")
                  for d in range(2)]
            hseq = [st.tile([128, 2 * NTOK], BF16, tag=f"hseq{i}", name=f"hseq{i}")
                    for i in range(4)]  # l0f, l0r, l1f, l1r

            SIG = mybir.ActivationFunctionType.Sigmoid
            TANH = mybir.ActivationFunctionType.Tanh

            def recurrence(layer):
                hs = [hseq[layer * 2], hseq[layer * 2 + 1]]
                c = [st.tile([128, 16], F32, tag=f"c{layer}{d}", name=f"c{layer}{d}")
                         for d in range(2)]
                h0 = st.tile([128, 16], BF16, tag=f"h0_{layer}")
                nc.vector.memset(h0[:], 0.0)
                for d in range(2):
                    nc.vector.memset(c[d][:], 0.0)

                def step(d, i):
                    t = i if d == 0 else S - 1 - i
                    wbase = (layer * 2 + d) * 2048
                    ps = psC.tile([128, 64], F32, tag="rec")
                    for m in range(8):
                        for kc in range(2):
                            if i == 0:
                                rhs = h0[:, kc * 8:kc * 8 + 8]
                            else:
                                tp = t - 1 if d == 0 else t + 1
                                rhs = hs[d][:, kc * NTOK + tp * BL:
                                            kc * NTOK + tp * BL + 8]
                            lt = whh[:, wbase + kc * 1024 + m * 128:
                                     wbase + kc * 1024 + m * 128 + 128]
                            nc.tensor.matmul(ps[:, m * 8:m * 8 + 8], lt, rhs,
                                             start=(kc == 0), stop=(kc == 1))
                    pre = wk.tile([128, 64], F32, tag=f"pre{d}")
                    xg_ap = xg[d].rearrange("p (m n) -> p m n", m=8)
                    nc.vector.tensor_add(pre[:], ps[:],
                                         xg_ap[:, :, t * BL:t * BL + 8])
                    sg = wk.tile([128, 48], F32, tag=f"sg{d}")
                    tg = wk.tile([128, 16], F32, tag=f"tg{d}")
                    nc.scalar.activation(sg[:], pre[:, 0:48], SIG)
                    nc.scalar.activation(tg[:], pre[:, 48:64], TANH)
                    tmp = wk.tile([128, 16], F32, tag=f"tm{d}")
                    nc.vector.tensor_mul(tmp[:], sg[:, 0:16], tg[:])
                    nc.vector.tensor_mul(c[d][:], c[d][:], sg[:, 16:32])
                    nc.vector.tensor_add(c[d][:], c[d][:], tmp[:])
                    tc_ = wk.tile([128, 16], F32, tag=f"tc{d}")
                    nc.scalar.activation(tc_[:], c[d][:], TANH)
                    h_ap = hs[d].rearrange("p (k n) -> p k n", k=2)
                    nc.vector.tensor_mul(h_ap[:, :, t * BL:t * BL + 8],
                                         sg[:, 32:48], tc_[:])

                for i in range(S):
                    step(0, i)
                    step(1, i)

            proj(wih0, xt, xg, 0)
            recurrence(0)
            x1_tiles = [hseq[0][:, 0:NTOK], hseq[0][:, NTOK:2 * NTOK],
                        hseq[1][:, 0:NTOK], hseq[1][:, NTOK:2 * NTOK]]
            proj(wih1, x1_tiles, xg, 16)
            recurrence(1)

            # ---- emissions: [5, NTOK] ----
            em = st.tile([5, NTOK], F32, tag="em")
            for n in range(2):
                ps = psB.tile([5, 512], F32, tag="proj")
                for kc in range(4):
                    lt = outw[:, kc * 8:kc * 8 + 5]
                    rhs = hseq[2 + kc // 2][:, (kc % 2) * NTOK + n * 512:
                                            (kc % 2) * NTOK + n * 512 + 512]
                    nc.tensor.matmul(ps[:], lt, rhs, start=(kc == 0),
                                     stop=(kc == 3))
                nc.vector.tensor_scalar_add(em[:, n * 512:(n + 1) * 512],
                                            ps[:], outb[:])
            nc.sync.dma_start(d_dbg_em[:], em[:])

            # ---- CRF ----
            EXP = mybir.ActivationFunctionType.Exp
            LN = mybir.ActivationFunctionType.Ln
            expem = st.tile([5, NTOK], F32, tag="expem")
            nc.scalar.activation(expem[:], em[:], EXP)
            # expem_eff = 1 + emmask*(expem-1)
            nc.vector.tensor_scalar_add(expem[:], expem[:], -1.0)
            nc.vector.tensor_mul(expem[:], expem[:], emmask[:])
            nc.vector.tensor_scalar_add(expem[:], expem[:], 1.0)

            a = st.tile([5, BL], F32, tag="alpha")
            # a0 = exp(em[:,0:BL] + start)
            nc.scalar.activation(a[:], em[:, 0:BL], EXP, bias=crfc[:, 5:6])
            zinvbuf = st.tile([1, 33 * BL], F32, tag="zinv")
            nc.vector.memset(zinvbuf[:], 1.0)

            nrounds = 0
            for t in range(1, S):
                pa = psT.tile([5, BL], F32, tag="crf")
                nc.tensor.matmul(pa[:], crfc[:, 0:5], a[:], start=True,
                                 stop=True)
                upd = wk.tile([5, BL], F32, tag="upd")
                nc.vector.tensor_mul(upd[:], pa[:],
                                     expem[:, t * BL:t * BL + BL])
                # a = a + m*(upd - a)
                dlt = wk.tile([5, BL], F32, tag="dlt")
                nc.vector.tensor_sub(dlt[:], upd[:], a[:])
                nc.vector.tensor_mul(dlt[:], dlt[:],
                                     emmask[:, t * BL:t * BL + BL])
                nc.vector.tensor_add(a[:], a[:], dlt[:])
                if t % 4 == 3:
                    r = nrounds
                    nrounds += 1
                    pz = psT.tile([1, BL], F32, tag="crf")
                    nc.tensor.matmul(pz[:], crfc[:, 7:8], a[:], start=True,
                                     stop=True)
                    zi_ap = zinvbuf.rearrange("p (b r) -> p b r", b=BL)
                    nc.vector.reciprocal(zi_ap[:, :, r:r + 1], pz[:])
                    pr = psT.tile([5, BL], F32, tag="crf")
                    nc.tensor.matmul(pr[:], ones15[:],
                                     zi_ap[:, :, r:r + 1].rearrange(
                                         "p b r -> p (b r)"),
                                     start=True, stop=True)
                    nc.vector.tensor_mul(a[:], a[:], pr[:])
            # final: a *= exp(end); zfin
            nc.vector.tensor_scalar_mul(a[:], a[:], crfc[:, 6:7])
            pz = psT.tile([1, BL], F32, tag="crf")
            nc.tensor.matmul(pz[:], crfc[:, 7:8], a[:], start=True, stop=True)
            lnzf = wk.tile([1, BL], F32, tag="lnzf")
            nc.scalar.activation(lnzf[:], pz[:], LN)
            lnzi = st.tile([1, 33 * BL], F32, tag="lnzi")
            nc.scalar.activation(lnzi[:], zinvbuf[:], LN)
            slzi = wk.tile([1, BL], F32, tag="slzi")
            li_ap = lnzi.rearrange("p (b r) -> p b r", b=BL)
            nc.vector.tensor_reduce(slzi[:], li_ap[:], mybir.AxisListType.X,
                                    mybir.AluOpType.add)
            # num_em
            emoh = st.tile([5, NTOK], F32, tag="emoh")
            nc.vector.tensor_mul(emoh[:], em[:], ohmask[:])
            emsum = wk.tile([5, BL], F32, tag="emsum")
            eo_ap = emoh.rearrange("p (t b) -> p b t", b=BL)
            nc.vector.tensor_reduce(emsum[:], eo_ap[:], mybir.AxisListType.X,
                                    mybir.AluOpType.add)
            pne = psT.tile([1, BL], F32, tag="crf")
            nc.tensor.matmul(pne[:], crfc[:, 7:8], emsum[:], start=True,
                             stop=True)
            # llh = numc + num_em - (ln zfin - sum ln zinv)
            llh = wk.tile([1, BL], F32, tag="llh")
            nc.vector.tensor_add(llh[:], pne[:], numc[:])
            nc.vector.tensor_sub(llh[:], llh[:], lnzf[:])
            nc.vector.tensor_add(llh[:], llh[:], slzi[:])
            nc.sync.dma_start(d_llh[:], llh[:])

    nc.compile()
    return nc


def prep_inputs(word_ids, char_ids, labels, lengths, word_emb, char_emb,
                conv_w3, conv_b3, conv_w4, conv_b4, conv_w5, conv_b5,
                out_w, out_b, crf_start, crf_end, crf_trans,
                lstm_Wih_l0f, lstm_Whh_l0f, lstm_b_l0f,
                lstm_Wih_l0r, lstm_Whh_l0r, lstm_b_l0r,
                lstm_Wih_l1f, lstm_Whh_l1f, lstm_b_l1f,
                lstm_Wih_l1r, lstm_Whh_l1r, lstm_b_l1r):
    perm = _gate_perm()

    def pack_wih(w, pad_to):
        wp = np.zeros((pad_to, 4 * HID), np.float32)
        w = np.asarray(w, np.float32)
        if w.shape[0] == COMB:  # layer 0: scatter char rows to aligned slots
            wp[0:300] = w[0:300]
            wp[320:370] = w[300:350]
            wp[384:434] = w[350:400]
            wp[448:498] = w[400:450]
        else:
            wp[:w.shape[0]] = w
        wp = wp[:, perm]
        # [128, 4K chunks * (8 m-chunks * 128)] -> [128, 4096]
        out = np.zeros((128, 4096), np.float32)
        for kc in range(4):
            out[:, kc * 1024:(kc + 1) * 1024] = wp[kc * 128:(kc + 1) * 128, :]
        return out

    def pack_whh(w):
        wp = np.asarray(w, np.float32)[:, perm]
        out = np.zeros((128, 2048), np.float32)
        for kc in range(2):
            out[:, kc * 1024:(kc + 1) * 1024] = wp[kc * 128:(kc + 1) * 128, :]
        return out

    wih0 = np.concatenate([pack_wih(lstm_Wih_l0f, 512),
                           pack_wih(lstm_Wih_l0r, 512)], axis=1)
    wih1 = np.concatenate([pack_wih(lstm_Wih_l1f, 512),
                           pack_wih(lstm_Wih_l1r, 512)], axis=1)
    whh = np.concatenate([pack_whh(lstm_Whh_l0f), pack_whh(lstm_Whh_l0r),
                          pack_whh(lstm_Whh_l1f), pack_whh(lstm_Whh_l1r)],
                         axis=1)
    biases = np.zeros((128, 32), np.float32)
    for col, b in enumerate([lstm_b_l0f, lstm_b_l0r, lstm_b_l1f, lstm_b_l1r]):
        bp = np.asarray(b, np.float32)[perm]
        biases[:, col * 8:(col + 1) * 8] = bp.reshape(8, 128).T

    outw_p = np.zeros((128, 32), np.float32)
    ow = np.asarray(out_w, np.float32)  # [512, 5]
    for kc in range(4):
        outw_p[:, kc * 8:kc * 8 + 5] = ow[kc * 128:(kc + 1) * 128, :]

    convA = np.zeros((100, 320), np.float32)
    convB = np.zeros((50, 128), np.float32)
    w3 = np.asarray(conv_w3, np.float32)  # [3,50,50]
    w4 = np.asarray(conv_w4, np.float32)
    w5 = np.asarray(conv_w5, np.float32)
    convA[:, 0:50] = w3[0:2].reshape(100, 50)
    convB[:, 0:50] = w3[2]
    convA[:, 64:114] = w4[0:2].reshape(100, 50)
    convA[:, 128:178] = w4[2:4].reshape(100, 50)
    convA[:, 192:242] = w5[0:2].reshape(100, 50)
    convA[:, 256:306] = w5[2:4].reshape(100, 50)
    convB[:, 64:114] = w5[4]
    convbias = np.stack([np.asarray(conv_b3, np.float32),
                         np.asarray(conv_b4, np.float32),
                         np.asarray(conv_b5, np.float32)], axis=1)

    crfc = np.zeros((5, 16), np.float32)
    ct = np.asarray(crf_trans, np.float32)
    crfc[:, 0:5] = np.exp(ct)
    crfc[:, 5] = np.asarray(crf_start, np.float32)
    crfc[:, 6] = np.exp(np.asarray(crf_end, np.float32))
    crfc[:, 7] = 1.0
    ones15 = np.ones((1, 5), np.float32)

    word_emb = np.asarray(word_emb, np.float32)
    char_emb = np.asarray(char_emb, np.float32)
    word_ids = np.asarray(word_ids)
    char_ids = np.asarray(char_ids)
    labels = np.asarray(labels)
    lengths = np.asarray(lengths)

    in_maps = []
    for c in range(NCORES):
        sl = slice(c * BL, (c + 1) * BL)
        wi = word_ids[sl]  # [BL, S]
        ci = char_ids[sl]  # [BL, S, W]
        tg = labels[sl]
        ln = lengths[sl]

        # xwe [384, NTOK], col = t*BL + b
        we = word_emb[wi]  # [BL, S, 300]
        xwe = np.zeros((384, NTOK), np.float32)
        xwe[0:300] = we.transpose(2, 1, 0).reshape(300, NTOK)

        # ce2 [100, NW*NTOK]: rows 0:50 = ce at char-pos q=w-2,
        # rows 50:100 = ce at q=w-1
        ce = char_emb[ci]  # [BL, S, W, 50]
        ce_f = ce.transpose(3, 2, 1, 0).reshape(50, W, NTOK)  # [50, q, tok]
        ce2 = np.zeros((100, NW * NTOK), np.float32)
        for w in range(NW):
            q0, q1 = w - 2, w - 1
            if 0 <= q0 < W:
                ce2[0:50, w * NTOK:(w + 1) * NTOK] = ce_f[:, q0]
            if 0 <= q1 < W:
                ce2[50:100, w * NTOK:(w + 1) * NTOK] = ce_f[:, q1]

        mask = (np.arange(S)[None, :] < ln[:, None])  # [BL, S]
        emmask = np.broadcast_to(
            mask.T.reshape(1, NTOK), (5, NTOK)).astype(np.float32)
        oh = (tg[:, :, None] == np.arange(5)[None, None, :])  # [BL,S,5]
        ohm = oh * mask[:, :, None]
        ohm[:, 0] = oh[:, 0]  # t=0 always counted
        ohmask = ohm.transpose(2, 1, 0).reshape(5, NTOK).astype(np.float32)

        numc = np.asarray(crf_start, np.float32)[tg[:, 0]].copy()
        tr = np.asarray(ct)[tg[:, :-1], tg[:, 1:]]  # [BL, S-1]
        numc += (tr * mask[:, 1:]).sum(axis=1)
        last = np.take_along_axis(tg, (ln - 1)[:, None], axis=1)[:, 0]
        numc += np.asarray(crf_end, np.float32)[last]

        in_maps.append({
            "ce2": ce2.astype(NP_BF16),
            "xwe": xwe.astype(NP_BF16),
            "convA": convA.astype(NP_BF16),
            "convB": convB.astype(NP_BF16),
            "convbias": convbias,
            "wih0": wih0.astype(NP_BF16),
            "wih1": wih1.astype(NP_BF16),
            "whh": whh.astype(NP_BF16),
            "bias": biases,
            "outw": outw_p.astype(NP_BF16),
            "outb": np.asarray(out_b, np.float32).reshape(5, 1),
            "crfc": crfc,
            "emmask": emmask.astype(NP_BF16),
            "ohmask": ohmask,
            "numc": numc.reshape(1, BL),
            "ones15": ones15,
        })
    return in_maps


def kernel(**inputs):
    if "nc" not in _CACHE:
        _CACHE["nc"] = build_nc()
    nc = _CACHE["nc"]
    in_maps = prep_inputs(**inputs)
    res = run_bass_kernel_spmd(nc, in_maps, core_ids=list(range(NCORES)))
    llh = np.concatenate([res.results[c]["llh"].reshape(-1)
                          for c in range(NCORES)])
    _CACHE["last_results"] = res
    return np.float32(-llh.mean())


# revision 9
# speedup vs baseline: 42.5831x; 42.5831x over previous
import sys

sys.path.insert(0, "/opt/trn_rl_repo")

import numpy as np

import concourse.bass as bass
import concourse.tile as tile
from concourse import bacc, mybir
from concourse.bass_utils import run_bass_kernel_spmd

# Model dims (hardcoded per contract)
B, S, W = 64, 128, 16
CDIM, NF = 50, 50
WDIM = 300
KS = [3, 4, 5]
HID, NLAB = 256, 5
COMB = WDIM + NF * 3  # 450
NCORES = 8
BL = B // NCORES  # 8 seqs per core
NTOK = BL * S  # 1024 tokens per core, col index = t*BL + b
NW = 21  # padded char positions (-2..18)

F32 = mybir.dt.float32
BF16 = mybir.dt.bfloat16
NP_BF16 = mybir.dt.np(BF16)

_CACHE = {}


def _gate_perm():
    # torch gate order i,f,g,o -> our chunk order i0 i1 f0 f1 o0 o1 g0 g1
    idx = np.arange(4 * HID).reshape(4, HID)  # i,f,g,o
    return np.concatenate([idx[0], idx[1], idx[3], idx[2]])  # i,f,o,g


def build_nc(skip=()):
    nc = bacc.Bacc("TRN2", target_bir_lowering=False, debug=False,
                   num_devices=NCORES)

    # ---- DRAM inputs ----
    d_ce2 = nc.dram_tensor("ce2", [100, NW * NTOK], BF16, kind="ExternalInput")
    d_xwe = nc.dram_tensor("xwe", [384, NTOK], BF16, kind="ExternalInput")
    d_convA = nc.dram_tensor("convA", [100, 5 * 64], BF16, kind="ExternalInput")
    d_convB = nc.dram_tensor("convB", [50, 2 * 64], BF16, kind="ExternalInput")
    d_convbias = nc.dram_tensor("convbias", [50, 3], F32, kind="ExternalInput")
    d_wih0 = nc.dram_tensor("wih0", [128, 2 * 4096], BF16, kind="ExternalInput")
    d_wih1 = nc.dram_tensor("wih1", [128, 2 * 4096], BF16, kind="ExternalInput")
    d_whh = nc.dram_tensor("whh", [128, 4 * 2048], BF16, kind="ExternalInput")
    d_bias = nc.dram_tensor("bias", [128, 4 * 8], F32, kind="ExternalInput")
    d_outw = nc.dram_tensor("outw", [128, 4 * 8], BF16, kind="ExternalInput")
    d_outb = nc.dram_tensor("outb", [5, 1], F32, kind="ExternalInput")
    d_crf = nc.dram_tensor("crfc", [5, 16], F32, kind="ExternalInput")
    # crfc cols: 0-4 expT rows(K=5,M=5), 5 start, 6 exp(end), 7 ones
    d_emmask = nc.dram_tensor("emmask", [5, NTOK], BF16, kind="ExternalInput")
    d_ohmask = nc.dram_tensor("ohmask", [5, NTOK], F32, kind="ExternalInput")
    d_numc = nc.dram_tensor("numc", [1, BL], F32, kind="ExternalInput")
    d_ones15 = nc.dram_tensor("ones15", [1, 5], F32, kind="ExternalInput")

    d_llh = nc.dram_tensor("llh", [1, BL], F32, kind="ExternalOutput")
    d_dbg_em = nc.dram_tensor("dbg_em", [5, NTOK], F32, kind="ExternalOutput")

    with tile.TileContext(nc) as tc:
        with (
            tc.tile_pool(name="big", bufs=1) as big,
            tc.tile_pool(name="wpool", bufs=1) as wp,
            tc.tile_pool(name="state", bufs=1) as st,
            tc.tile_pool(name="work", bufs=3) as wk,
            tc.tile_pool(name="pacc", bufs=2) as pk,
            tc.tile_pool(name="psB", bufs=2, space="PSUM") as psB,
        ):
            # ---- load phase ----
            ce2 = big.tile([100, NW * NTOK], BF16, tag="ce2")
            nc.sync.dma_start(ce2[:], d_ce2[:])
            xt = [big.tile([128, NTOK], BF16, tag=f"x{i}", name=f"x{i}")
                  for i in range(4)]
            for i in range(3):
                nc.sync.dma_start(xt[i][:], d_xwe[i * 128:(i + 1) * 128, :])
            nc.vector.memset(xt[3][:], 0.0)

            convA = wp.tile([100, 5 * 64], BF16, tag="cA")
            convB = wp.tile([50, 2 * 64], BF16, tag="cB")
            convbias = wp.tile([50, 3], F32, tag="cb")
            nc.sync.dma_start(convA[:], d_convA[:])
            nc.sync.dma_start(convB[:], d_convB[:])
            nc.sync.dma_start(convbias[:], d_convbias[:])

            wih0 = wp.tile([128, 2 * 4096], BF16, tag="wih0")
            wih1 = wp.tile([128, 2 * 4096], BF16, tag="wih1")
            whh = wp.tile([128, 4 * 2048], BF16, tag="whh")
            bias = wp.tile([128, 4 * 8], F32, tag="bias")
            outw = wp.tile([128, 4 * 8], BF16, tag="outw")
            outb = wp.tile([5, 1], F32, tag="outb")
            crfc = wp.tile([5, 16], F32, tag="crfc")
            emmask = wp.tile([5, NTOK], BF16, tag="emmask")
            ohmask = wp.tile([5, NTOK], F32, tag="ohmask")
            numc = wp.tile([1, BL], F32, tag="numc")
            ones15 = wp.tile([1, 5], F32, tag="ones15")
            for dst, src in [(wih0, d_wih0), (wih1, d_wih1), (whh, d_whh),
                             (bias, d_bias), (outw, d_outw), (outb, d_outb),
                             (crfc, d_crf), (emmask, d_emmask),
                             (ohmask, d_ohmask), (numc, d_numc),
                             (ones15, d_ones15)]:
                nc.sync.dma_start(dst[:], src[:])

            # ---- char conv: 3 kernels, max over positions ----
            psA_cm = tc.tile_pool(name="psA", bufs=4, space="PSUM")
            psA = psA_cm.__enter__()
            acc = [pk.tile([50, NTOK], F32, tag=f"acc{k}", name=f"acc{k}")
                   for k in range(3)]
            for a in acc:
                nc.vector.memset(a[:], 0.0)

            # (kernel_idx, n_positions, list of (lhsT_sel, q_offset))
            # lhsT_sel: ('A', col, rows) or ('B', col, rows)
            plans = [
                (0, 16, [(('A', 0, 100), -1), (('B', 0, 50), 1)]),
                (1, 17, [(('A', 64, 100), -2), (('A', 128, 100), 0)]),
                (2, 16, [(('A', 192, 100), -2), (('A', 256, 100), 0),
                         (('B', 64, 50), 2)]),
            ]
            for ki, npos, taps in (() if "conv" in skip else plans):
                for p in range(npos):
                    for n in range(2):
                        ps = psA.tile([50, 512], F32, tag="cps")
                        for ti, (sel, qoff) in enumerate(taps):
                            which, col, rows = sel
                            lhsT = (convA if which == 'A' else convB)
                            lt = lhsT[0:rows, col:col + 50]
                            w = p + qoff + 2
                            rhs = ce2[0:rows, w * NTOK + n * 512:
                                      w * NTOK + n * 512 + 512]
                            nc.tensor.matmul(ps[:], lt, rhs,
                                             start=(ti == 0),
                                             stop=(ti == len(taps) - 1))
                        nc.vector.tensor_max(acc[ki][:, n * 512:(n + 1) * 512],
                                             acc[ki][:, n * 512:(n + 1) * 512],
                                             ps[:])
            # relu(acc + b) -> aligned char rows: k3->t2[64:114],
            # k4->t3[0:50], k5->t3[64:114] (Wih rows permuted to match)
            RL = mybir.ActivationFunctionType.Relu
            nc.scalar.activation(xt[2][64:114, :], acc[0][:],
                                 RL, bias=convbias[:, 0:1])
            nc.scalar.activation(xt[3][0:50, :], acc[1][:],
                                 RL, bias=convbias[:, 1:2])
            nc.scalar.activation(xt[3][64:114, :], acc[2][:],
                                 RL, bias=convbias[:, 2:3])
            psA_cm.__exit__(None, None, None)
            psC_cm = tc.tile_pool(name="psC", bufs=4, space="PSUM")
            psC = psC_cm.__enter__()
            psT_cm = tc.tile_pool(name="psT", bufs=2, space="PSUM")
            psT = psT_cm.__enter__()

            # ---- helpers ----
            def proj(wih_t, src_tiles, xg_d, bias_col0):
                # xg[d][:, m*NTOK + t*BL+b] for d in 0,1
                for d in range(2):
                    for m in range(8):
                        for n in range(2):
                            ps = psB.tile([128, 512], F32, tag="proj")
                            for kc in range(4):
                                lt = wih_t[:, d * 4096 + kc * 1024 + m * 128:
                                           d * 4096 + kc * 1024 + m * 128 + 128]
                                rhs = src_tiles[kc][:, n * 512:(n + 1) * 512]
                                nc.tensor.matmul(ps[:], lt, rhs,
                                                 start=(kc == 0), stop=(kc == 3))
                            nc.vector.tensor_scalar_add(
                                xg_d[d][:, m * NTOK + n * 512:
                                        m * NTOK + n * 512 + 512],
                                ps[:],
                                bias[:, bias_col0 + d * 8 + m:
                                     bias_col0 + d * 8 + m + 1])

            xg = [big.tile([128, 8 * NTOK], BF16, tag=f"xg{d}", name=f"xg{d}")
                  for d in range(2)]
            hseq = [st.tile([128, 2 * NTOK], BF16, tag=f"hseq{i}", name=f"hseq{i}")
                    for i in range(4)]  # l0f, l0r, l1f, l1r

            SIG = mybir.ActivationFunctionType.Sigmoid
            TANH = mybir.ActivationFunctionType.Tanh

            def recurrence(layer):
                hs = [hseq[layer * 2], hseq[layer * 2 + 1]]
                c = [st.tile([128, 16], F32, tag=f"c{layer}{d}", name=f"c{layer}{d}")
                         for d in range(2)]
                h0 = st.tile([128, 16], BF16, tag=f"h0_{layer}")
                nc.vector.memset(h0[:], 0.0)
                for d in range(2):
                    nc.vector.memset(c[d][:], 0.0)

                def step(d, i):
                    t = i if d == 0 else S - 1 - i
                    wbase = (layer * 2 + d) * 2048
                    ps = psC.tile([128, 64], F32, tag="rec")
                    for m in range(8):
                        for kc in range(2):
                            if i == 0:
                                rhs = h0[:, kc * 8:kc * 8 + 8]
                            else:
                                tp = t - 1 if d == 0 else t + 1
                                rhs = hs[d][:, kc * NTOK + tp * BL:
                                            kc * NTOK + tp * BL + 8]
                            lt = whh[:, wbase + kc * 1024 + m * 128:
                                     wbase + kc * 1024 + m * 128 + 128]
                            nc.tensor.matmul(ps[:, m * 8:m * 8 + 8], lt, rhs,
                                             start=(kc == 0), stop=(kc == 1))
                    pre = wk.tile([128, 64], F32, tag=f"pre{d}")
                    xg_ap = xg[d].rearrange("p (m n) -> p m n", m=8)
                    nc.vector.tensor_add(pre[:], ps[:],
                                         xg_ap[:, :, t * BL:t * BL + 8])
                    sg = wk.tile([128, 48], F32, tag=f"sg{d}")
                    tg = wk.tile([128, 16], F32, tag=f"tg{d}")
                    nc.scalar.activation(sg[:], pre[:, 0:48], SIG)
                    nc.scalar.activation(tg[:], pre[:, 48:64], TANH)
                    tmp = wk.tile([128, 16], F32, tag=f"tm{d}")
                    nc.vector.tensor_mul(tmp[:], sg[:, 0:16], tg[:])
                    nc.vector.tensor_mul(c[d][:], c[d][:], sg[:, 16:32])
                    nc.vector.tensor_add(c[d][:], c[d][:], tmp[:])
                    tc_ = wk.tile([128, 16], F32, tag=f"tc{d}")
                    nc.scalar.activation(tc_[:], c[d][:], TANH)
                    h_ap = hs[d].rearrange("p (k n) -> p k n", k=2)
                    nc.vector.tensor_mul(h_ap[:, :, t * BL:t * BL + 8],
                                         sg[:, 32:48], tc_[:])

                for i in range(S):
                    step(0, i)
                    step(1, i)

            if "rec" in skip:
                for hh in hseq:
                    nc.vector.memset(hh[:], 0.0)
            proj(wih0, xt, xg, 0)
            if "rec" not in skip:
                recurrence(0)
            x1_tiles = [hseq[0][:, 0:NTOK], hseq[0][:, NTOK:2 * NTOK],
                        hseq[1][:, 0:NTOK], hseq[1][:, NTOK:2 * NTOK]]
            proj(wih1, x1_tiles, xg, 16)
            if "rec" not in skip:
                recurrence(1)

            # ---- emissions: [5, NTOK] ----
            em = st.tile([5, NTOK], F32, tag="em")
            for n in range(2):
                ps = psB.tile([5, 512], F32, tag="proj")
                for kc in range(4):
                    lt = outw[:, kc * 8:kc * 8 + 5]
                    rhs = hseq[2 + kc // 2][:, (kc % 2) * NTOK + n * 512:
                                            (kc % 2) * NTOK + n * 512 + 512]
                    nc.tensor.matmul(ps[:], lt, rhs, start=(kc == 0),
                                     stop=(kc == 3))
                nc.vector.tensor_scalar_add(em[:, n * 512:(n + 1) * 512],
                                            ps[:], outb[:])
            nc.sync.dma_start(d_dbg_em[:], em[:])

            # ---- CRF ----
            EXP = mybir.ActivationFunctionType.Exp
            LN = mybir.ActivationFunctionType.Ln
            expem = st.tile([5, NTOK], F32, tag="expem")
            nc.scalar.activation(expem[:], em[:], EXP)
            # expem_eff = 1 + emmask*(expem-1)
            nc.vector.tensor_scalar_add(expem[:], expem[:], -1.0)
            nc.vector.tensor_mul(expem[:], expem[:], emmask[:])
            nc.vector.tensor_scalar_add(expem[:], expem[:], 1.0)

            a = st.tile([5, BL], F32, tag="alpha")
            # a0 = exp(em[:,0:BL] + start)
            nc.scalar.activation(a[:], em[:, 0:BL], EXP, bias=crfc[:, 5:6])
            zinvbuf = st.tile([1, 33 * BL], F32, tag="zinv")
            nc.vector.memset(zinvbuf[:], 1.0)

            nrounds = 0
            for t in ([] if "crf" in skip else range(1, S)):
                pa = psT.tile([5, BL], F32, tag="crf")
                nc.tensor.matmul(pa[:], crfc[:, 0:5], a[:], start=True,
                                 stop=True)
                if t < 64:
                    # lengths >= S//2, no masking needed: a = upd
                    nc.vector.tensor_mul(a[:], pa[:],
                                         expem[:, t * BL:t * BL + BL])
                else:
                    upd = wk.tile([5, BL], F32, tag="upd")
                    nc.vector.tensor_mul(upd[:], pa[:],
                                         expem[:, t * BL:t * BL + BL])
                    # a = a + m*(upd - a)
                    dlt = wk.tile([5, BL], F32, tag="dlt")
                    nc.vector.tensor_sub(dlt[:], upd[:], a[:])
                    nc.vector.tensor_mul(dlt[:], dlt[:],
                                         emmask[:, t * BL:t * BL + BL])
                    nc.vector.tensor_add(a[:], a[:], dlt[:])
                if t % 8 == 7:
                    r = nrounds
                    nrounds += 1
                    pz = psT.tile([1, BL], F32, tag="crf")
                    nc.tensor.matmul(pz[:], crfc[:, 7:8], a[:], start=True,
                                     stop=True)
                    zi_ap = zinvbuf.rearrange("p (b r) -> p b r", b=BL)
                    nc.vector.reciprocal(zi_ap[:, :, r:r + 1], pz[:])
                    pr = psT.tile([5, BL], F32, tag="crf")
                    nc.tensor.matmul(pr[:], ones15[:],
                                     zi_ap[:, :, r:r + 1].rearrange(
                                         "p b r -> p (b r)"),
                                     start=True, stop=True)
                    nc.vector.tensor_mul(a[:], a[:], pr[:])
            # final: a *= exp(end); zfin
            nc.vector.tensor_scalar_mul(a[:], a[:], crfc[:, 6:7])
            pz = psT.tile([1, BL], F32, tag="crf")
            nc.tensor.matmul(pz[:], crfc[:, 7:8], a[:], start=True, stop=True)
            lnzf = wk.tile([1, BL], F32, tag="lnzf")
            nc.scalar.activation(lnzf[:], pz[:], LN)
            lnzi = st.tile([1, 33 * BL], F32, tag="lnzi")
            nc.scalar.activation(lnzi[:], zinvbuf[:], LN)
            slzi = wk.tile([1, BL], F32, tag="slzi")
            li_ap = lnzi.rearrange("p (b r) -> p b r", b=BL)
            nc.vector.tensor_reduce(slzi[:], li_ap[:], mybir.AxisListType.X,
                                    mybir.AluOpType.add)
            # num_em
            emoh = st.tile([5, NTOK], F32, tag="emoh")
            nc.vector.tensor_mul(emoh[:], em[:], ohmask[:])
            emsum = wk.tile([5, BL], F32, tag="emsum")
            eo_ap = emoh.rearrange("p (t b) -> p b t", b=BL)
            nc.vector.tensor_reduce(emsum[:], eo_ap[:], mybir.AxisListType.X,
                                    mybir.AluOpType.add)
            pne = psT.tile([1, BL], F32, tag="crf")
            nc.tensor.matmul(pne[:], crfc[:, 7:8], emsum[:], start=True,
                             stop=True)
            # llh = numc + num_em - (ln zfin - sum ln zinv)
            llh = wk.tile([1, BL], F32, tag="llh")
            nc.vector.tensor_add(llh[:], pne[:], numc[:])
            nc.vector.tensor_sub(llh[:], llh[:], lnzf[:])
            nc.vector.tensor_add(llh[:], llh[:], slzi[:])
            nc.sync.dma_start(d_llh[:], llh[:])
            psT_cm.__exit__(None, None, None)
            psC_cm.__exit__(None, None, None)

    nc.compile()
    return nc


def prep_inputs(word_ids, char_ids, labels, lengths, word_emb, char_emb,
                conv_w3, conv_b3, conv_w4, conv_b4, conv_w5, conv_b5,
                out_w, out_b, crf_start, crf_end, crf_trans,
                lstm_Wih_l0f, lstm_Whh_l0f, lstm_b_l0f,
                lstm_Wih_l0r, lstm_Whh_l0r, lstm_b_l0r,
                lstm_Wih_l1f, lstm_Whh_l1f, lstm_b_l1f,
                lstm_Wih_l1r, lstm_Whh_l1r, lstm_b_l1r):
    perm = _gate_perm()

    def pack_wih(w, pad_to):
        wp = np.zeros((pad_to, 4 * HID), np.float32)
        w = np.asarray(w, np.float32)
        if w.shape[0] == COMB:  # layer 0: scatter char rows to aligned slots
            wp[0:300] = w[0:300]
            wp[320:370] = w[300:350]
            wp[384:434] = w[350:400]
            wp[448:498] = w[400:450]
        else:
            wp[:w.shape[0]] = w
        wp = wp[:, perm]
        # [128, 4K chunks * (8 m-chunks * 128)] -> [128, 4096]
        out = np.zeros((128, 4096), np.float32)
        for kc in range(4):
            out[:, kc * 1024:(kc + 1) * 1024] = wp[kc * 128:(kc + 1) * 128, :]
        return out

    def pack_whh(w):
        wp = np.asarray(w, np.float32)[:, perm]
        out = np.zeros((128, 2048), np.float32)
        for kc in range(2):
            out[:, kc * 1024:(kc + 1) * 1024] = wp[kc * 128:(kc + 1) * 128, :]
        return out

    wih0 = np.concatenate([pack_wih(lstm_Wih_l0f, 512),
                           pack_wih(lstm_Wih_l0r, 512)], axis=1)
    wih1 = np.concatenate([pack_wih(lstm_Wih_l1f, 512),
                           pack_wih(lstm_Wih_l1r, 512)], axis=1)
    whh = np.concatenate([pack_whh(lstm_Whh_l0f), pack_whh(lstm_Whh_l0r),
                          pack_whh(lstm_Whh_l1f), pack_whh(lstm_Whh_l1r)],
                         axis=1)
    biases = np.zeros((128, 32), np.float32)
    for col, b in enumerate([lstm_b_l0f, lstm_b_l0r, lstm_b_l1f, lstm_b_l1r]):
        bp = np.asarray(b, np.float32)[perm]
        biases[:, col * 8:(col + 1) * 8] = bp.reshape(8, 128).T

    outw_p = np.zeros((128, 32), np.float32)
    ow = np.asarray(out_w, np.float32)  # [512, 5]
    for kc in range(4):
        outw_p[:, kc * 8:kc * 8 + 5] = ow[kc * 128:(kc + 1) * 128, :]

    convA = np.zeros((100, 320), np.float32)
    convB = np.zeros((50, 128), np.float32)
    w3 = np.asarray(conv_w3, np.float32)  # [3,50,50]
    w4 = np.asarray(conv_w4, np.float32)
    w5 = np.asarray(conv_w5, np.float32)
    convA[:, 0:50] = w3[0:2].reshape(100, 50)
    convB[:, 0:50] = w3[2]
    convA[:, 64:114] = w4[0:2].reshape(100, 50)
    convA[:, 128:178] = w4[2:4].reshape(100, 50)
    convA[:, 192:242] = w5[0:2].reshape(100, 50)
    convA[:, 256:306] = w5[2:4].reshape(100, 50)
    convB[:, 64:114] = w5[4]
    convbias = np.stack([np.asarray(conv_b3, np.float32),
                         np.asarray(conv_b4, np.float32),
                         np.asarray(conv_b5, np.float32)], axis=1)

    crfc = np.zeros((5, 16), np.float32)
    ct = np.asarray(crf_trans, np.float32)
    crfc[:, 0:5] = np.exp(ct)
    crfc[:, 5] = np.asarray(crf_start, np.float32)
    crfc[:, 6] = np.exp(np.asarray(crf_end, np.float32))
    crfc[:, 7] = 1.0
    ones15 = np.ones((1, 5), np.float32)

    word_emb = np.asarray(word_emb, np.float32)
    char_emb = np.asarray(char_emb, np.float32)
    word_ids = np.asarray(word_ids)
    char_ids = np.asarray(char_ids)
    labels = np.asarray(labels)
    lengths = np.asarray(lengths)

    in_maps = []
    for c in range(NCORES):
        sl = slice(c * BL, (c + 1) * BL)
        wi = word_ids[sl]  # [BL, S]
        ci = char_ids[sl]  # [BL, S, W]
        tg = labels[sl]
        ln = lengths[sl]

        # xwe [384, NTOK], col = t*BL + b
        we = word_emb[wi]  # [BL, S, 300]
        xwe = np.zeros((384, NTOK), np.float32)
        xwe[0:300] = we.transpose(2, 1, 0).reshape(300, NTOK)

        # ce2 [100, NW*NTOK]: rows 0:50 = ce at char-pos q=w-2,
        # rows 50:100 = ce at q=w-1
        ce = char_emb[ci]  # [BL, S, W, 50]
        ce_f = ce.transpose(3, 2, 1, 0).reshape(50, W, NTOK)  # [50, q, tok]
        ce2 = np.zeros((100, NW * NTOK), np.float32)
        for w in range(NW):
            q0, q1 = w - 2, w - 1
            if 0 <= q0 < W:
                ce2[0:50, w * NTOK:(w + 1) * NTOK] = ce_f[:, q0]
            if 0 <= q1 < W:
                ce2[50:100, w * NTOK:(w + 1) * NTOK] = ce_f[:, q1]

        mask = (np.arange(S)[None, :] < ln[:, None])  # [BL, S]
        emmask = np.broadcast_to(
            mask.T.reshape(1, NTOK), (5, NTOK)).astype(np.float32)
        oh = (tg[:, :, None] == np.arange(5)[None, None, :])  # [BL,S,5]
        ohm = oh * mask[:, :, None]
        ohm[:, 0] = oh[:, 0]  # t=0 always counted
        ohmask = ohm.transpose(2, 1, 0).reshape(5, NTOK).astype(np.float32)

        numc = np.asarray(crf_start, np.float32)[tg[:, 0]].copy()
        tr = np.asarray(ct)[tg[:, :-1], tg[:, 1:]]  # [BL, S-1]
        numc += (tr * mask[:, 1:]).sum(axis=1)
        last = np.take_along_axis(tg, (ln - 1)[:, None], axis=1)[:, 0]
        numc += np.asarray(crf_end, np.float32)[last]

        in_maps.append({
            "ce2": ce2.astype(NP_BF16),
            "xwe": xwe.astype(NP_BF16),
            "convA": convA.astype(NP_BF16),
            "convB": convB.astype(NP_BF16),
            "convbias": convbias,
            "wih0": wih0.astype(NP_BF16),
            "wih1": wih1.astype(NP_BF16),
            "whh": whh.astype(NP_BF16),
            "bias": biases,
            "outw": outw_p.astype(NP_BF16),
            "outb": np.asarray(out_b, np.float32).reshape(5, 1),
            "crfc": crfc,
            "emmask": emmask.astype(NP_BF16),
            "ohmask": ohmask,
            "numc": numc.reshape(1, BL),
            "ones15": ones15,
        })
    return in_maps


def kernel(**inputs):
    if "nc" not in _CACHE:
        _CACHE["nc"] = build_nc()
    nc = _CACHE["nc"]
    in_maps = prep_inputs(**inputs)
    res = run_bass_kernel_spmd(nc, in_maps, core_ids=list(range(NCORES)))
    llh = np.concatenate([res.results[c]["llh"].reshape(-1)
                          for c in range(NCORES)])
    _CACHE["last_results"] = res
    return np.float32(-llh.mean())


# revision 10
# speedup vs baseline: 44.5670x; 1.0466x over previous
import sys

sys.path.insert(0, "/opt/trn_rl_repo")

import numpy as np

import concourse.bass as bass
import concourse.tile as tile
from concourse import bacc, mybir
from concourse.bass_utils import run_bass_kernel_spmd

# Model dims (hardcoded per contract)
B, S, W = 64, 128, 16
CDIM, NF = 50, 50
WDIM = 300
KS = [3, 4, 5]
HID, NLAB = 256, 5
COMB = WDIM + NF * 3  # 450
NCORES = 8
BL = B // NCORES  # 8 seqs per core
NTOK = BL * S  # 1024 tokens per core, col index = t*BL + b
NW = 21  # padded char positions (-2..18)

F32 = mybir.dt.float32
BF16 = mybir.dt.bfloat16
NP_BF16 = mybir.dt.np(BF16)

_CACHE = {}


def _gate_perm():
    # torch gate order i,f,g,o -> our chunk order i0 i1 f0 f1 o0 o1 g0 g1
    idx = np.arange(4 * HID).reshape(4, HID)  # i,f,g,o
    return np.concatenate([idx[0], idx[1], idx[3], idx[2]])  # i,f,o,g


def build_nc(skip=()):
    nc = bacc.Bacc("TRN2", target_bir_lowering=False, debug=False,
                   num_devices=NCORES)

    # ---- DRAM inputs ----
    d_ce2 = nc.dram_tensor("ce2", [100, NW * NTOK], BF16, kind="ExternalInput")
    d_xwe = nc.dram_tensor("xwe", [384, NTOK], BF16, kind="ExternalInput")
    d_convA = nc.dram_tensor("convA", [100, 5 * 64], BF16, kind="ExternalInput")
    d_convB = nc.dram_tensor("convB", [50, 2 * 64], BF16, kind="ExternalInput")
    d_convbias = nc.dram_tensor("convbias", [50, 3], F32, kind="ExternalInput")
    d_wih0 = nc.dram_tensor("wih0", [128, 2 * 4096], BF16, kind="ExternalInput")
    d_wih1 = nc.dram_tensor("wih1", [128, 2 * 4096], BF16, kind="ExternalInput")
    d_whh = nc.dram_tensor("whh", [128, 4 * 2048], BF16, kind="ExternalInput")
    d_bias = nc.dram_tensor("bias", [128, 4 * 8], F32, kind="ExternalInput")
    d_outw = nc.dram_tensor("outw", [128, 4 * 8], BF16, kind="ExternalInput")
    d_outb = nc.dram_tensor("outb", [5, 1], F32, kind="ExternalInput")
    d_crf = nc.dram_tensor("crfc", [5, 16], F32, kind="ExternalInput")
    # crfc cols: 0-4 expT rows(K=5,M=5), 5 start, 6 exp(end), 7 ones
    d_emmask = nc.dram_tensor("emmask", [5, NTOK], BF16, kind="ExternalInput")
    d_ohmask = nc.dram_tensor("ohmask", [5, NTOK], F32, kind="ExternalInput")
    d_numc = nc.dram_tensor("numc", [1, BL], F32, kind="ExternalInput")
    d_ones15 = nc.dram_tensor("ones15", [1, 5], F32, kind="ExternalInput")

    d_llh = nc.dram_tensor("llh", [1, BL], F32, kind="ExternalOutput")
    d_dbg_em = nc.dram_tensor("dbg_em", [5, NTOK], F32, kind="ExternalOutput")

    with tile.TileContext(nc) as tc:
        with (
            tc.tile_pool(name="big", bufs=1) as big,
            tc.tile_pool(name="wpool", bufs=1) as wp,
            tc.tile_pool(name="state", bufs=1) as st,
            tc.tile_pool(name="work", bufs=3) as wk,
            tc.tile_pool(name="pacc", bufs=2) as pk,
            tc.tile_pool(name="psB", bufs=2, space="PSUM") as psB,
        ):
            # ---- load phase ----
            ce2 = big.tile([100, NW * NTOK], BF16, tag="ce2")
            nc.sync.dma_start(ce2[:], d_ce2[:])
            xt = [big.tile([128, NTOK], BF16, tag=f"x{i}", name=f"x{i}")
                  for i in range(4)]
            for i in range(3):
                nc.sync.dma_start(xt[i][:], d_xwe[i * 128:(i + 1) * 128, :])
            nc.vector.memset(xt[3][:], 0.0)

            convA = wp.tile([100, 5 * 64], BF16, tag="cA")
            convB = wp.tile([50, 2 * 64], BF16, tag="cB")
            convbias = wp.tile([50, 3], F32, tag="cb")
            nc.sync.dma_start(convA[:], d_convA[:])
            nc.sync.dma_start(convB[:], d_convB[:])
            nc.sync.dma_start(convbias[:], d_convbias[:])

            wih0 = wp.tile([128, 2 * 4096], BF16, tag="wih0")
            wih1 = wp.tile([128, 2 * 4096], BF16, tag="wih1")
            whh = wp.tile([128, 4 * 2048], BF16, tag="whh")
            bias = wp.tile([128, 4 * 8], F32, tag="bias")
            outw = wp.tile([128, 4 * 8], BF16, tag="outw")
            outb = wp.tile([5, 1], F32, tag="outb")
            crfc = wp.tile([5, 16], F32, tag="crfc")
            emmask = wp.tile([5, NTOK], BF16, tag="emmask")
            ohmask = wp.tile([5, NTOK], F32, tag="ohmask")
            numc = wp.tile([1, BL], F32, tag="numc")
            ones15 = wp.tile([1, 5], F32, tag="ones15")
            for dst, src in [(wih0, d_wih0), (wih1, d_wih1), (whh, d_whh),
                             (bias, d_bias), (outw, d_outw), (outb, d_outb),
                             (crfc, d_crf), (emmask, d_emmask),
                             (ohmask, d_ohmask), (numc, d_numc),
                             (ones15, d_ones15)]:
                nc.sync.dma_start(dst[:], src[:])

            # ---- char conv: 3 kernels, max over positions ----
            psA_cm = tc.tile_pool(name="psA", bufs=4, space="PSUM")
            psA = psA_cm.__enter__()
            acc = [pk.tile([50, NTOK], F32, tag=f"acc{k}", name=f"acc{k}")
                   for k in range(3)]
            for a in acc:
                nc.vector.memset(a[:], 0.0)

            # (kernel_idx, n_positions, list of (lhsT_sel, q_offset))
            # lhsT_sel: ('A', col, rows) or ('B', col, rows)
            plans = [
                (0, 16, [(('A', 0, 100), -1), (('B', 0, 50), 1)]),
                (1, 17, [(('A', 64, 100), -2), (('A', 128, 100), 0)]),
                (2, 16, [(('A', 192, 100), -2), (('A', 256, 100), 0),
                         (('B', 64, 50), 2)]),
            ]
            for ki, npos, taps in (() if "conv" in skip else plans):
                for p in range(npos):
                    for n in range(2):
                        ps = psA.tile([50, 512], F32, tag="cps")
                        for ti, (sel, qoff) in enumerate(taps):
                            which, col, rows = sel
                            lhsT = (convA if which == 'A' else convB)
                            lt = lhsT[0:rows, col:col + 50]
                            w = p + qoff + 2
                            rhs = ce2[0:rows, w * NTOK + n * 512:
                                      w * NTOK + n * 512 + 512]
                            nc.tensor.matmul(ps[:], lt, rhs,
                                             start=(ti == 0),
                                             stop=(ti == len(taps) - 1))
                        nc.vector.tensor_max(acc[ki][:, n * 512:(n + 1) * 512],
                                             acc[ki][:, n * 512:(n + 1) * 512],
                                             ps[:])
            # relu(acc + b) -> aligned char rows: k3->t2[64:114],
            # k4->t3[0:50], k5->t3[64:114] (Wih rows permuted to match)
            RL = mybir.ActivationFunctionType.Relu
            nc.scalar.activation(xt[2][64:114, :], acc[0][:],
                                 RL, bias=convbias[:, 0:1])
            nc.scalar.activation(xt[3][0:50, :], acc[1][:],
                                 RL, bias=convbias[:, 1:2])
            nc.scalar.activation(xt[3][64:114, :], acc[2][:],
                                 RL, bias=convbias[:, 2:3])
            psA_cm.__exit__(None, None, None)
            psC_cm = tc.tile_pool(name="psC", bufs=4, space="PSUM")
            psC = psC_cm.__enter__()
            psT_cm = tc.tile_pool(name="psT", bufs=2, space="PSUM")
            psT = psT_cm.__enter__()

            # ---- helpers ----
            def proj(wih_t, src_tiles, xg_d, bias_col0):
                # xg[d][:, m*NTOK + t*BL+b] for d in 0,1
                for d in range(2):
                    for m in range(8):
                        for n in range(2):
                            ps = psB.tile([128, 512], F32, tag="proj")
                            for kc in range(4):
                                lt = wih_t[:, d * 4096 + kc * 1024 + m * 128:
                                           d * 4096 + kc * 1024 + m * 128 + 128]
                                rhs = src_tiles[kc][:, n * 512:(n + 1) * 512]
                                nc.tensor.matmul(ps[:], lt, rhs,
                                                 start=(kc == 0), stop=(kc == 3))
                            nc.vector.tensor_scalar_add(
                                xg_d[d][:, m * NTOK + n * 512:
                                        m * NTOK + n * 512 + 512],
                                ps[:],
                                bias[:, bias_col0 + d * 8 + m:
                                     bias_col0 + d * 8 + m + 1])

            xg = [big.tile([128, 8 * NTOK], BF16, tag=f"xg{d}", name=f"xg{d}")
                  for d in range(2)]
            hseq = [st.tile([128, 2 * NTOK], BF16, tag=f"hseq{i}", name=f"hseq{i}")
                    for i in range(4)]  # l0f, l0r, l1f, l1r

            SIG = mybir.ActivationFunctionType.Sigmoid
            TANH = mybir.ActivationFunctionType.Tanh

            def recurrence(layer):
                hs = [hseq[layer * 2], hseq[layer * 2 + 1]]
                c = [st.tile([128, 16], F32, tag=f"c{layer}{d}", name=f"c{layer}{d}")
                         for d in range(2)]
                h0 = st.tile([128, 16], BF16, tag=f"h0_{layer}")
                nc.vector.memset(h0[:], 0.0)
                for d in range(2):
                    nc.vector.memset(c[d][:], 0.0)

                def step(d, i):
                    t = i if d == 0 else S - 1 - i
                    wbase = (layer * 2 + d) * 2048
                    ps = psC.tile([128, 64], F32, tag="rec")
                    for m in range(8):
                        for kc in range(2):
                            if i == 0:
                                rhs = h0[:, kc * 8:kc * 8 + 8]
                            else:
                                tp = t - 1 if d == 0 else t + 1
                                rhs = hs[d][:, kc * NTOK + tp * BL:
                                            kc * NTOK + tp * BL + 8]
                            lt = whh[:, wbase + kc * 1024 + m * 128:
                                     wbase + kc * 1024 + m * 128 + 128]
                            nc.tensor.matmul(ps[:, m * 8:m * 8 + 8], lt, rhs,
                                             start=(kc == 0), stop=(kc == 1))
                    pre = wk.tile([128, 64], F32, tag=f"pre{d}")
                    xg_ap = xg[d].rearrange("p (m n) -> p m n", m=8)
                    nc.vector.tensor_add(pre[:], ps[:],
                                         xg_ap[:, :, t * BL:t * BL + 8])
                    sg = wk.tile([128, 48], F32, tag=f"sg{d}")
                    tg = wk.tile([128, 16], F32, tag=f"tg{d}")
                    # sigmoid(z) = 0.5*tanh(z/2)+0.5: keep ACT on one LUT
                    nc.scalar.activation(sg[:], pre[:, 0:48], TANH, scale=0.5)
                    nc.vector.tensor_scalar(sg[:], sg[:], 0.5, 0.5,
                                            mybir.AluOpType.mult,
                                            mybir.AluOpType.add)
                    nc.scalar.activation(tg[:], pre[:, 48:64], TANH)
                    tmp = wk.tile([128, 16], F32, tag=f"tm{d}")
                    nc.vector.tensor_mul(tmp[:], sg[:, 0:16], tg[:])
                    nc.vector.tensor_mul(c[d][:], c[d][:], sg[:, 16:32])
                    nc.vector.tensor_add(c[d][:], c[d][:], tmp[:])
                    tc_ = wk.tile([128, 16], F32, tag=f"tc{d}")
                    nc.scalar.activation(tc_[:], c[d][:], TANH)
                    h_ap = hs[d].rearrange("p (k n) -> p k n", k=2)
                    nc.vector.tensor_mul(h_ap[:, :, t * BL:t * BL + 8],
                                         sg[:, 32:48], tc_[:])

                for i in range(S):
                    step(0, i)
                    step(1, i)

            if "rec" in skip:
                for hh in hseq:
                    nc.vector.memset(hh[:], 0.0)
            proj(wih0, xt, xg, 0)
            if "rec" not in skip:
                recurrence(0)
            x1_tiles = [hseq[0][:, 0:NTOK], hseq[0][:, NTOK:2 * NTOK],
                        hseq[1][:, 0:NTOK], hseq[1][:, NTOK:2 * NTOK]]
            proj(wih1, x1_tiles, xg, 16)
            if "rec" not in skip:
                recurrence(1)

            # ---- emissions: [5, NTOK] ----
            em = st.tile([5, NTOK], F32, tag="em")
            for n in range(2):
                ps = psB.tile([5, 512], F32, tag="proj")
                for kc in range(4):
                    lt = outw[:, kc * 8:kc * 8 + 5]
                    rhs = hseq[2 + kc // 2][:, (kc % 2) * NTOK + n * 512:
                                            (kc % 2) * NTOK + n * 512 + 512]
                    nc.tensor.matmul(ps[:], lt, rhs, start=(kc == 0),
                                     stop=(kc == 3))
                nc.vector.tensor_scalar_add(em[:, n * 512:(n + 1) * 512],
                                            ps[:], outb[:])
            nc.sync.dma_start(d_dbg_em[:], em[:])

            # ---- CRF ----
            EXP = mybir.ActivationFunctionType.Exp
            LN = mybir.ActivationFunctionType.Ln
            expem = st.tile([5, NTOK], F32, tag="expem")
            nc.scalar.activation(expem[:], em[:], EXP)
            # expem_eff = 1 + emmask*(expem-1)
            nc.vector.tensor_scalar_add(expem[:], expem[:], -1.0)
            nc.vector.tensor_mul(expem[:], expem[:], emmask[:])
            nc.vector.tensor_scalar_add(expem[:], expem[:], 1.0)

            a = st.tile([5, BL], F32, tag="alpha")
            # a0 = exp(em[:,0:BL] + start)
            nc.scalar.activation(a[:], em[:, 0:BL], EXP, bias=crfc[:, 5:6])
            zinvbuf = st.tile([1, 33 * BL], F32, tag="zinv")
            nc.vector.memset(zinvbuf[:], 1.0)

            nrounds = 0
            for t in ([] if "crf" in skip else range(1, S)):
                pa = psT.tile([5, BL], F32, tag="crf")
                nc.tensor.matmul(pa[:], crfc[:, 0:5], a[:], start=True,
                                 stop=True)
                if t < 64:
                    # lengths >= S//2, no masking needed: a = upd
                    nc.vector.tensor_mul(a[:], pa[:],
                                         expem[:, t * BL:t * BL + BL])
                else:
                    upd = wk.tile([5, BL], F32, tag="upd")
                    nc.vector.tensor_mul(upd[:], pa[:],
                                         expem[:, t * BL:t * BL + BL])
                    # a = a + m*(upd - a)
                    dlt = wk.tile([5, BL], F32, tag="dlt")
                    nc.vector.tensor_sub(dlt[:], upd[:], a[:])
                    nc.vector.tensor_mul(dlt[:], dlt[:],
                                         emmask[:, t * BL:t * BL + BL])
                    nc.vector.tensor_add(a[:], a[:], dlt[:])
                if t % 8 == 7:
                    r = nrounds
                    nrounds += 1
                    pz = psT.tile([1, BL], F32, tag="crf")
                    nc.tensor.matmul(pz[:], crfc[:, 7:8], a[:], start=True,
                                     stop=True)
                    zi_ap = zinvbuf.rearrange("p (b r) -> p b r", b=BL)
                    nc.vector.reciprocal(zi_ap[:, :, r:r + 1], pz[:])
                    pr = psT.tile([5, BL], F32, tag="crf")
                    nc.tensor.matmul(pr[:], ones15[:],
                                     zi_ap[:, :, r:r + 1].rearrange(
                                         "p b r -> p (b r)"),
                                     start=True, stop=True)
                    nc.vector.tensor_mul(a[:], a[:], pr[:])
            # final: a *= exp(end); zfin
            nc.vector.tensor_scalar_mul(a[:], a[:], crfc[:, 6:7])
            pz = psT.tile([1, BL], F32, tag="crf")
            nc.tensor.matmul(pz[:], crfc[:, 7:8], a[:], start=True, stop=True)
            lnzf = wk.tile([1, BL], F32, tag="lnzf")
            nc.scalar.activation(lnzf[:], pz[:], LN)
            lnzi = st.tile([1, 33 * BL], F32, tag="lnzi")
            nc.scalar.activation(lnzi[:], zinvbuf[:], LN)
            slzi = wk.tile([1, BL], F32, tag="slzi")
            li_ap = lnzi.rearrange("p (b r) -> p b r", b=BL)
            nc.vector.tensor_reduce(slzi[:], li_ap[:], mybir.AxisListType.X,
                                    mybir.AluOpType.add)
            # num_em
            emoh = st.tile([5, NTOK], F32, tag="emoh")
            nc.vector.tensor_mul(emoh[:], em[:], ohmask[:])
            emsum = wk.tile([5, BL], F32, tag="emsum")
            eo_ap = emoh.rearrange("p (t b) -> p b t", b=BL)
            nc.vector.tensor_reduce(emsum[:], eo_ap[:], mybir.AxisListType.X,
                                    mybir.AluOpType.add)
            pne = psT.tile([1, BL], F32, tag="crf")
            nc.tensor.matmul(pne[:], crfc[:, 7:8], emsum[:], start=True,
                             stop=True)
            # llh = numc + num_em - (ln zfin - sum ln zinv)
            llh = wk.tile([1, BL], F32, tag="llh")
            nc.vector.tensor_add(llh[:], pne[:], numc[:])
            nc.vector.tensor_sub(llh[:], llh[:], lnzf[:])
            nc.vector.tensor_add(llh[:], llh[:], slzi[:])
            nc.sync.dma_start(d_llh[:], llh[:])
            psT_cm.__exit__(None, None, None)
            psC_cm.__exit__(None, None, None)

    nc.compile()
    return nc


def prep_inputs(word_ids, char_ids, labels, lengths, word_emb, char_emb,
                conv_w3, conv_b3, conv_w4, conv_b4, conv_w5, conv_b5,
                out_w, out_b, crf_start, crf_end, crf_trans,
                lstm_Wih_l0f, lstm_Whh_l0f, lstm_b_l0f,
                lstm_Wih_l0r, lstm_Whh_l0r, lstm_b_l0r,
                lstm_Wih_l1f, lstm_Whh_l1f, lstm_b_l1f,
                lstm_Wih_l1r, lstm_Whh_l1r, lstm_b_l1r):
    perm = _gate_perm()

    def pack_wih(w, pad_to):
        wp = np.zeros((pad_to, 4 * HID), np.float32)
        w = np.asarray(w, np.float32)
        if w.shape[0] == COMB:  # layer 0: scatter char rows to aligned slots
            wp[0:300] = w[0:300]
            wp[320:370] = w[300:350]
            wp[384:434] = w[350:400]
            wp[448:498] = w[400:450]
        else:
            wp[:w.shape[0]] = w
        wp = wp[:, perm]
        # [128, 4K chunks * (8 m-chunks * 128)] -> [128, 4096]
        out = np.zeros((128, 4096), np.float32)
        for kc in range(4):
            out[:, kc * 1024:(kc + 1) * 1024] = wp[kc * 128:(kc + 1) * 128, :]
        return out

    def pack_whh(w):
        wp = np.asarray(w, np.float32)[:, perm]
        out = np.zeros((128, 2048), np.float32)
        for kc in range(2):
            out[:, kc * 1024:(kc + 1) * 1024] = wp[kc * 128:(kc + 1) * 128, :]
        return out

    wih0 = np.concatenate([pack_wih(lstm_Wih_l0f, 512),
                           pack_wih(lstm_Wih_l0r, 512)], axis=1)
    wih1 = np.concatenate([pack_wih(lstm_Wih_l1f, 512),
                           pack_wih(lstm_Wih_l1r, 512)], axis=1)
    whh = np.concatenate([pack_whh(lstm_Whh_l0f), pack_whh(lstm_Whh_l0r),
                          pack_whh(lstm_Whh_l1f), pack_whh(lstm_Whh_l1r)],
                         axis=1)
    biases = np.zeros((128, 32), np.float32)
    for col, b in enumerate([lstm_b_l0f, lstm_b_l0r, lstm_b_l1f, lstm_b_l1r]):
        bp = np.asarray(b, np.float32)[perm]
        biases[:, col * 8:(col + 1) * 8] = bp.reshape(8, 128).T

    outw_p = np.zeros((128, 32), np.float32)
    ow = np.asarray(out_w, np.float32)  # [512, 5]
    for kc in range(4):
        outw_p[:, kc * 8:kc * 8 + 5] = ow[kc * 128:(kc + 1) * 128, :]

    convA = np.zeros((100, 320), np.float32)
    convB = np.zeros((50, 128), np.float32)
    w3 = np.asarray(conv_w3, np.float32)  # [3,50,50]
    w4 = np.asarray(conv_w4, np.float32)
    w5 = np.asarray(conv_w5, np.float32)
    convA[:, 0:50] = w3[0:2].reshape(100, 50)
    convB[:, 0:50] = w3[2]
    convA[:, 64:114] = w4[0:2].reshape(100, 50)
    convA[:, 128:178] = w4[2:4].reshape(100, 50)
    convA[:, 192:242] = w5[0:2].reshape(100, 50)
    convA[:, 256:306] = w5[2:4].reshape(100, 50)
    convB[:, 64:114] = w5[4]
    convbias = np.stack([np.asarray(conv_b3, np.float32),
                         np.asarray(conv_b4, np.float32),
                         np.asarray(conv_b5, np.float32)], axis=1)

    crfc = np.zeros((5, 16), np.float32)
    ct = np.asarray(crf_trans, np.float32)
    crfc[:, 0:5] = np.exp(ct)
    crfc[:, 5] = np.asarray(crf_start, np.float32)
    crfc[:, 6] = np.exp(np.asarray(crf_end, np.float32))
    crfc[:, 7] = 1.0
    ones15 = np.ones((1, 5), np.float32)

    word_emb = np.asarray(word_emb, np.float32)
    char_emb = np.asarray(char_emb, np.float32)
    word_ids = np.asarray(word_ids)
    char_ids = np.asarray(char_ids)
    labels = np.asarray(labels)
    lengths = np.asarray(lengths)

    in_maps = []
    for c in range(NCORES):
        sl = slice(c * BL, (c + 1) * BL)
        wi = word_ids[sl]  # [BL, S]
        ci = char_ids[sl]  # [BL, S, W]
        tg = labels[sl]
        ln = lengths[sl]

        # xwe [384, NTOK], col = t*BL + b
        we = word_emb[wi]  # [BL, S, 300]
        xwe = np.zeros((384, NTOK), np.float32)
        xwe[0:300] = we.transpose(2, 1, 0).reshape(300, NTOK)

        # ce2 [100, NW*NTOK]: rows 0:50 = ce at char-pos q=w-2,
        # rows 50:100 = ce at q=w-1
        ce = char_emb[ci]  # [BL, S, W, 50]
        ce_f = ce.transpose(3, 2, 1, 0).reshape(50, W, NTOK)  # [50, q, tok]
        ce2 = np.zeros((100, NW * NTOK), np.float32)
        for w in range(NW):
            q0, q1 = w - 2, w - 1
            if 0 <= q0 < W:
                ce2[0:50, w * NTOK:(w + 1) * NTOK] = ce_f[:, q0]
            if 0 <= q1 < W:
                ce2[50:100, w * NTOK:(w + 1) * NTOK] = ce_f[:, q1]

        mask = (np.arange(S)[None, :] < ln[:, None])  # [BL, S]
        emmask = np.broadcast_to(
            mask.T.reshape(1, NTOK), (5, NTOK)).astype(np.float32)
        oh = (tg[:, :, None] == np.arange(5)[None, None, :])  # [BL,S,5]
        ohm = oh * mask[:, :, None]
        ohm[:, 0] = oh[:, 0]  # t=0 always counted
        ohmask = ohm.transpose(2, 1, 0).reshape(5, NTOK).astype(np.float32)

        numc = np.asarray(crf_start, np.float32)[tg[:, 0]].copy()
        tr = np.asarray(ct)[tg[:, :-1], tg[:, 1:]]  # [BL, S-1]
        numc += (tr * mask[:, 1:]).sum(axis=1)
        last = np.take_along_axis(tg, (ln - 1)[:, None], axis=1)[:, 0]
        numc += np.asarray(crf_end, np.float32)[last]

        in_maps.append({
            "ce2": ce2.astype(NP_BF16),
            "xwe": xwe.astype(NP_BF16),
            "convA": convA.astype(NP_BF16),
            "convB": convB.astype(NP_BF16),
            "convbias": convbias,
            "wih0": wih0.astype(NP_BF16),
            "wih1": wih1.astype(NP_BF16),
            "whh": whh.astype(NP_BF16),
            "bias": biases,
            "outw": outw_p.astype(NP_BF16),
            "outb": np.asarray(out_b, np.float32).reshape(5, 1),
            "crfc": crfc,
            "emmask": emmask.astype(NP_BF16),
            "ohmask": ohmask,
            "numc": numc.reshape(1, BL),
            "ones15": ones15,
        })
    return in_maps


def kernel(**inputs):
    if "nc" not in _CACHE:
        _CACHE["nc"] = build_nc()
    nc = _CACHE["nc"]
    in_maps = prep_inputs(**inputs)
    res = run_bass_kernel_spmd(nc, in_maps, core_ids=list(range(NCORES)))
    llh = np.concatenate([res.results[c]["llh"].reshape(-1)
                          for c in range(NCORES)])
    _CACHE["last_results"] = res
    return np.float32(-llh.mean())
